# revision 18
# baseline (speedup 1.0000x reference)
"""NetVLAD forward on 8 Trainium2 NeuronCores.

Reference computation (per batch b):
    logits = conv_w @ x_flat[b]            # [K, N]    (1x1 conv, K=64, C=128, N=4096)
    a      = softmax(logits, axis=K)
    vlad   = a @ x_flat[b].T - sum_n(a) * centroids    # [K, C]
    vlad   = l2norm(vlad, axis=C)          # intra-normalize
    out[b] = l2norm(vlad.reshape(K*C))     # global normalize

Sharding: pure data-parallel over the batch dim (8 batches per core);
conv weight replicated.  No collectives needed.

Per-core dataflow, per batch (N = 4096 split into 32 chunks of 128):
  PE (8224 cyc/batch, the bottleneck):
    mm1   per chunk: pl[n,k]  = x_chunk[c,n].T @ conv_w.T[c,k]      (64 cyc)
    xpose per chunk: pt[n,c]  = transpose(x_chunk)  (bf16 psum)     (128 cyc)
    mm2   per chunk: pv[c,k] += xts_chunk[n,c].T @ a_chunk[n,k]     (64 cyc)
    asum  per chunk: pv[k,64]+= a_chunk[n,k].T @ r_col[n,1]         (1 cyc)
  ACT:  exp per 16-chunk wave (pl f32 psum -> e bf16 sbuf), 1/4 of copies
  DVE:  reduce_sum_k per wave, reciprocal, scale wave 0, 1/2 of copies
  Pool: scale wave 1 (broadcast tensor_tensor), 1/4 of copies
  copies: pt psum -> xts sbuf per 8-chunk wave (mm2's stationary operand)

The vlad comes out transposed ([C, K] in pv[:, 0:64]) with asum in
pv[0:64, 64]; the tiny per-batch epilogue (centroid subtraction + two L2
normalizations, ~0.4% of the FLOPs) runs on the host after the gather.

Softmax skips the max-subtraction: logits are ~N(0, 1.28), |logit| < 8 over
this input distribution, exp() is safely in fp32 range.
"""

import numpy as np
import ml_dtypes
from contextlib import ExitStack

import concourse.bass as bass
import concourse.bacc as bacc
import concourse.tile as tile
import concourse.mybir as mybir
from concourse import bass_utils

B, C, K = 64, 128, 64
HW = 64 * 64  # N = H*W
NCORES = 8
BPC = B // NCORES  # batches per core
F32 = mybir.dt.float32
BF16 = mybir.dt.bfloat16

NCHUNK = 128            # n-columns per chunk (PE partition limit)
NCH = HW // NCHUNK      # chunks per batch = 32
LWAVE = 16              # chunks per logits/exp wave (psum: [128,16,64] f32 = 2 banks)
NLW = NCH // LWAVE      # logit waves per batch = 2
TWAVE = 8               # chunks per transpose/copy wave ([128,8,128] bf16 = 1 bank)
NTW = NCH // TWAVE      # transpose waves per batch = 4

# engine assignment knobs (gpsimd cannot touch PSUM -> copies are ACT/DVE only)
# per pt-wave: list of (engine, lo_chunk, hi_chunk) psum->sbuf copy slices
COPY_PLAN = (
    (("scalar", 0, 8),),
    (("scalar", 0, 8),),
    (("vector", 0, 8),),
    (("vector", 0, 8),),
)
# (engine, lo_chunk, hi_chunk) softmax scale slices over all 32 chunks (sbuf only)
SCALE_PLAN = (("vector", 0, 4), ("gpsimd", 4, 11), ("vector", 11, 16))
DRAIN_ENG = "vector"               # pv psum -> sbuf


def _bcast_k(ap, k):
    """Broadcast a [128, W] AP over a trailing K axis (stride 0)."""
    return bass.AP(tensor=ap.tensor, offset=ap.offset, ap=[*ap.ap, [0, k]])


def _netvlad_tile(tc: tile.TileContext, out_d, x_d, w_d, ident_d):
    nc = tc.nc
    eng = {
        "scalar": nc.scalar,
        "vector": nc.vector,
        "gpsimd": nc.gpsimd,
    }
    with ExitStack() as ctx:
        const = ctx.enter_context(tc.tile_pool(name="const", bufs=1))
        xpool = ctx.enter_context(tc.tile_pool(name="x", bufs=4))
        epool = ctx.enter_context(tc.tile_pool(name="e", bufs=3))
        spool = ctx.enter_context(tc.tile_pool(name="s", bufs=3))
        apool = ctx.enter_context(tc.tile_pool(name="a", bufs=3))
        xtpool = ctx.enter_context(tc.tile_pool(name="xt", bufs=2))
        opool = ctx.enter_context(tc.tile_pool(name="o", bufs=2))
        pl_pool = ctx.enter_context(tc.tile_pool(name="pl", bufs=1, space="PSUM"))
        pt_pool = ctx.enter_context(tc.tile_pool(name="pt", bufs=2, space="PSUM"))
        pv_pool = ctx.enter_context(tc.tile_pool(name="pv", bufs=1, space="PSUM"))
        pvs_pool = ctx.enter_context(tc.tile_pool(name="pvs", bufs=1, space="PSUM"))

        w_sb = const.tile([C, K], BF16)
        nc.sync.dma_start(out=w_sb, in_=w_d)
        ident_sb = const.tile([C, C], BF16)
        nc.sync.dma_start(out=ident_sb, in_=ident_d)
        ones_sb = const.tile([C, 1], BF16)
        nc.gpsimd.memset(ones_sb, 1.0)

        NXC = HW // 2  # x load chunk: half a batch per DMA

        def issue_front(ib):
            """DMA loads, mm1 logits, softmax chain, transposes + copies."""
            xparts = []
            nparts = 4 if ib == 0 else 2
            psz = HW // nparts
            for h_ in range(nparts):
                xh = xpool.tile([C, psz], BF16, tag=f"xh{nparts}", name=f"xh{nparts}")
                nc.sync.dma_start(out=xh, in_=x_d[ib][:, h_ * psz : (h_ + 1) * psz])
                xparts.append(xh)

            def xsl(i):
                n0 = i * NCHUNK
                return xparts[n0 // psz][:, n0 % psz : n0 % psz + NCHUNK]

            # --- logits + softmax, two half-batch waves (shorter critical
            # chain: wave 0's exp/tree/scale overlap wave 1's mm1) ---
            a = apool.tile([C, NCH, K], BF16, tag="a")
            for w in range(2):
                lo_w = w * (NCH // 2)
                hi_w = lo_w + NCH // 2
                pl = pl_pool.tile([C, NCH // 2, K], F32, tag=f"pl{w}", name=f"pl{w}")
                for i in range(lo_w, hi_w):
                    nc.tensor.matmul(
                        pl[:, i - lo_w, :], lhsT=xsl(i), rhs=w_sb, start=True, stop=True
                    )
                e = epool.tile([C, NCH // 2, K], BF16, tag=f"e{w}", name=f"e{w}")
                nc.scalar.activation(e, pl, mybir.ActivationFunctionType.Exp)
                s4 = spool.tile([C, NCH // 2, 2], BF16, tag=f"s4{w}", name=f"s4{w}")
                with nc.allow_low_precision(reason="softmax sum, tolerance 2e-2"):
                    nc.vector.reduce_sum(
                        s4[:, :, 0], e, axis=mybir.AxisListType.X
                    )
                    nc.vector.tensor_copy(out=s4[:, :, 1], in_=s4[:, :, 0])
                s2 = s4
                # r2: reciprocal pairs keep the scale's broadcast AP packed (2x)
                r2 = spool.tile([C, NCH // 2, 2], BF16, tag=f"r2{w}", name=f"r2{w}")
                with nc.allow_low_precision(reason="softmax recip, tolerance 2e-2"):
                    nc.vector.reciprocal(r2, s2)
                for sname, lo, hi in SCALE_PLAN:
                    in1 = bass.AP(
                        tensor=r2.tensor,
                        offset=r2.offset + lo * r2.ap[1][0],
                        ap=[r2.ap[0], [r2.ap[1][0], hi - lo], [0, K // 2], [1, 2]],
                    )
                    eng[sname].tensor_tensor(
                        out=a[:, lo_w + lo : lo_w + hi, :],
                        in0=e[:, lo:hi, :],
                        in1=in1,
                        op=mybir.AluOpType.mult,
                    )

            # --- x transposes + psum->sbuf copies, in TWAVE-chunk waves ---
            xts = xtpool.tile([C, NCH, C], BF16, tag="xts")
            for tw in range(NTW):
                pt = pt_pool.tile([C, TWAVE, C], BF16, tag="pt")
                for j in range(TWAVE):
                    i = tw * TWAVE + j
                    nc.tensor.transpose(pt[:, j, :], in_=xsl(i), identity=ident_sb)
                for cname, lo, hi in COPY_PLAN[tw]:
                    dst = xts[:, tw * TWAVE + lo : tw * TWAVE + hi, :]
                    srcp = pt[:, lo:hi, :]
                    if cname == "scalar":
                        eng[cname].copy(out=dst, in_=srcp)
                    else:
                        eng[cname].tensor_copy(out=dst, in_=srcp)
            return a, xts

        def issue_back(ib, a, xts):
            """mm2 + asum accumulation and the psum drain for batch ib."""
            pv = pv_pool.tile([C, K], F32, tag="pv")
            pvs = pvs_pool.tile([K, 1], F32, tag="pvs")
            for i in range(NCH):
                a_chunk = a[:, i, :]
                nc.tensor.matmul(
                    pv,
                    lhsT=xts[:, i, :],
                    rhs=a_chunk,
                    start=(i == 0),
                    stop=(i == NCH - 1),
                )
                nc.tensor.matmul(
                    pvs,
                    lhsT=a_chunk,
                    rhs=ones_sb,
                    start=(i == 0),
                    stop=(i == NCH - 1),
                )
            outt = opool.tile([C, K + 1], F32, tag="o")
            if DRAIN_ENG == "scalar":
                nc.scalar.copy(out=outt[:, 0:K], in_=pv)
                nc.scalar.copy(out=outt[0:K, K : K + 1], in_=pvs)
            else:
                eng[DRAIN_ENG].tensor_copy(out=outt[:, 0:K], in_=pv)
                eng[DRAIN_ENG].tensor_copy(out=outt[0:K, K : K + 1], in_=pvs)
            nc.sync.dma_start(out=out_d[ib], in_=outt)

        pending = None
        for ib in range(BPC):
            front = issue_front(ib)
            if pending is not None:
                issue_back(ib - 1, *pending)
            pending = front
        issue_back(BPC - 1, *pending)


_NC_CACHE = None


def _get_nc():
    global _NC_CACHE
    if _NC_CACHE is None:
        nc = bacc.Bacc(
            "TRN2",
            target_bir_lowering=False,
            debug=False,
            num_devices=NCORES,
        )
        x_d = nc.dram_tensor("x", [BPC, C, HW], BF16, kind="ExternalInput").ap()
        w_d = nc.dram_tensor("w_t", [C, K], BF16, kind="ExternalInput").ap()
        ident_d = nc.dram_tensor("ident", [C, C], BF16, kind="ExternalInput").ap()
        out_d = nc.dram_tensor("out", [BPC, C, K + 1], F32, kind="ExternalOutput").ap()
        with tile.TileContext(nc) as tc:
            _netvlad_tile(tc, out_d, x_d, w_d, ident_d)
        nc.compile()
        _NC_CACHE = nc
    return _NC_CACHE


def _make_in_maps(x, conv_w):
    bf16 = ml_dtypes.bfloat16
    x_flat = np.ascontiguousarray(x.reshape(B, C, HW).astype(bf16))
    w_t = np.ascontiguousarray(conv_w.T.astype(bf16))  # [C, K]
    ident = np.eye(C, dtype=np.float32).astype(bf16)
    in_maps = []
    for core in range(NCORES):
        in_maps.append(
            {
                "x": x_flat[core * BPC : (core + 1) * BPC],
                "w_t": w_t,
                "ident": ident,
            }
        )
    return in_maps


def _run(in_maps, trace=False, **kwargs):
    nc = _get_nc()
    return bass_utils.run_bass_kernel_spmd(
        nc, in_maps, core_ids=list(range(NCORES)), trace=trace, **kwargs
    )


def _postprocess(raw, centroids):
    """raw: [B, C, K+1] = [vladT | asum] -> [B, K*C] normalized."""
    vlad = raw[:, :, :K].transpose(0, 2, 1) - raw[:, :K, K][:, :, None] * centroids[None]
    norms = np.sqrt((vlad * vlad).sum(axis=2, keepdims=True))
    vlad = vlad / np.maximum(norms, 1e-12)
    out = vlad.reshape(raw.shape[0], K * C)
    gn = np.sqrt((out * out).sum(axis=1, keepdims=True))
    return out / np.maximum(gn, 1e-12)


def kernel(x, conv_w, centroids):
    x = np.asarray(x)
    conv_w = np.asarray(conv_w)
    centroids = np.asarray(centroids, dtype=np.float32)
    res = _run(_make_in_maps(x, conv_w))
    raw = np.concatenate([r["out"] for r in res.results], axis=0)  # [B, C, K+1]
    return _postprocess(raw.astype(np.float32), centroids).astype(np.float32)


# revision 19
# speedup vs baseline: 1.0161x; 1.0161x over previous
"""NetVLAD forward on 8 Trainium2 NeuronCores.

Reference computation (per batch b):
    logits = conv_w @ x_flat[b]            # [K, N]    (1x1 conv, K=64, C=128, N=4096)
    a      = softmax(logits, axis=K)
    vlad   = a @ x_flat[b].T - sum_n(a) * centroids    # [K, C]
    vlad   = l2norm(vlad, axis=C)          # intra-normalize
    out[b] = l2norm(vlad.reshape(K*C))     # global normalize

Sharding: pure data-parallel over the batch dim (8 batches per core);
conv weight replicated.  No collectives needed.

Per-core dataflow, per batch (N = 4096 split into 32 chunks of 128):
  PE (8224 cyc/batch, the bottleneck):
    mm1   per chunk: pl[n,k]  = x_chunk[c,n].T @ conv_w.T[c,k]      (64 cyc)
    xpose per chunk: pt[n,c]  = transpose(x_chunk)  (bf16 psum)     (128 cyc)
    mm2   per chunk: pv[c,k] += xts_chunk[n,c].T @ a_chunk[n,k]     (64 cyc)
    asum  per chunk: pv[k,64]+= a_chunk[n,k].T @ r_col[n,1]         (1 cyc)
  ACT:  exp per 16-chunk wave (pl f32 psum -> e bf16 sbuf), 1/4 of copies
  DVE:  reduce_sum_k per wave, reciprocal, scale wave 0, 1/2 of copies
  Pool: scale wave 1 (broadcast tensor_tensor), 1/4 of copies
  copies: pt psum -> xts sbuf per 8-chunk wave (mm2's stationary operand)

The vlad comes out transposed ([C, K] in pv[:, 0:64]) with asum in
pv[0:64, 64]; the tiny per-batch epilogue (centroid subtraction + two L2
normalizations, ~0.4% of the FLOPs) runs on the host after the gather.

Softmax skips the max-subtraction: logits are ~N(0, 1.28), |logit| < 8 over
this input distribution, exp() is safely in fp32 range.
"""

import numpy as np
import ml_dtypes
from contextlib import ExitStack

import concourse.bass as bass
import concourse.bacc as bacc
import concourse.tile as tile
import concourse.mybir as mybir
from concourse import bass_utils

B, C, K = 64, 128, 64
HW = 64 * 64  # N = H*W
NCORES = 8
BPC = B // NCORES  # batches per core
F32 = mybir.dt.float32
BF16 = mybir.dt.bfloat16

NCHUNK = 128            # n-columns per chunk (PE partition limit)
NCH = HW // NCHUNK      # chunks per batch = 32
LWAVE = 16              # chunks per logits/exp wave (psum: [128,16,64] f32 = 2 banks)
NLW = NCH // LWAVE      # logit waves per batch = 2
TWAVE = 8               # chunks per transpose/copy wave ([128,8,128] bf16 = 1 bank)
NTW = NCH // TWAVE      # transpose waves per batch = 4

# engine assignment knobs (gpsimd cannot touch PSUM -> copies are ACT/DVE only)
# per pt-wave: list of (engine, lo_chunk, hi_chunk) psum->sbuf copy slices
COPY_PLAN = (
    (("scalar", 0, 8),),
    (("scalar", 0, 8),),
    (("vector", 0, 8),),
    (("vector", 0, 8),),
)
# (engine, lo_chunk, hi_chunk) softmax scale slices over all 32 chunks (sbuf only)
SCALE_PLAN = (("vector", 0, 3), ("gpsimd", 3, 13), ("vector", 13, 16))
DRAIN_ENG = "vector"               # pv psum -> sbuf


def _bcast_k(ap, k):
    """Broadcast a [128, W] AP over a trailing K axis (stride 0)."""
    return bass.AP(tensor=ap.tensor, offset=ap.offset, ap=[*ap.ap, [0, k]])


def _netvlad_tile(tc: tile.TileContext, out_d, x_d, w_d, ident_d):
    nc = tc.nc
    eng = {
        "scalar": nc.scalar,
        "vector": nc.vector,
        "gpsimd": nc.gpsimd,
    }
    with ExitStack() as ctx:
        const = ctx.enter_context(tc.tile_pool(name="const", bufs=1))
        xpool = ctx.enter_context(tc.tile_pool(name="x", bufs=4))
        epool = ctx.enter_context(tc.tile_pool(name="e", bufs=3))
        spool = ctx.enter_context(tc.tile_pool(name="s", bufs=3))
        apool = ctx.enter_context(tc.tile_pool(name="a", bufs=3))
        xtpool = ctx.enter_context(tc.tile_pool(name="xt", bufs=2))
        opool = ctx.enter_context(tc.tile_pool(name="o", bufs=2))
        pl_pool = ctx.enter_context(tc.tile_pool(name="pl", bufs=1, space="PSUM"))
        pt_pool = ctx.enter_context(tc.tile_pool(name="pt", bufs=2, space="PSUM"))
        pv_pool = ctx.enter_context(tc.tile_pool(name="pv", bufs=1, space="PSUM"))
        pvs_pool = ctx.enter_context(tc.tile_pool(name="pvs", bufs=1, space="PSUM"))

        w_sb = const.tile([C, K], BF16)
        nc.sync.dma_start(out=w_sb, in_=w_d)
        ident_sb = const.tile([C, C], BF16)
        nc.sync.dma_start(out=ident_sb, in_=ident_d)
        ones_sb = const.tile([C, 1], BF16)
        nc.gpsimd.memset(ones_sb, 1.0)

        NXC = HW // 2  # x load chunk: half a batch per DMA

        def issue_front(ib):
            """DMA loads, mm1 logits, softmax chain, transposes + copies."""
            xparts = []
            nparts = 4 if ib == 0 else 2
            psz = HW // nparts
            for h_ in range(nparts):
                xh = xpool.tile([C, psz], BF16, tag=f"xh{nparts}", name=f"xh{nparts}")
                nc.sync.dma_start(out=xh, in_=x_d[ib][:, h_ * psz : (h_ + 1) * psz])
                xparts.append(xh)

            def xsl(i):
                n0 = i * NCHUNK
                return xparts[n0 // psz][:, n0 % psz : n0 % psz + NCHUNK]

            # --- logits + softmax, two half-batch waves (shorter critical
            # chain: wave 0's exp/tree/scale overlap wave 1's mm1) ---
            a = apool.tile([C, NCH, K], BF16, tag="a")
            for w in range(2):
                lo_w = w * (NCH // 2)
                hi_w = lo_w + NCH // 2
                pl = pl_pool.tile([C, NCH // 2, K], F32, tag=f"pl{w}", name=f"pl{w}")
                for i in range(lo_w, hi_w):
                    nc.tensor.matmul(
                        pl[:, i - lo_w, :], lhsT=xsl(i), rhs=w_sb, start=True, stop=True
                    )
                e = epool.tile([C, NCH // 2, K], BF16, tag=f"e{w}", name=f"e{w}")
                nc.scalar.activation(e, pl, mybir.ActivationFunctionType.Exp)
                # sum over K via pairwise-add tree: tensor_tensor gets the DVE
                # 2x mode (0.52ns/elem), tensor_reduce does not (1.04)
                h = spool.tile([C, NCH // 2, K // 2], BF16, tag=f"h{w}", name=f"h{w}")
                with nc.allow_low_precision(reason="softmax sum, tolerance 2e-2"):
                    nc.vector.tensor_tensor(
                        out=h,
                        in0=e[:, :, 0 : K // 2],
                        in1=e[:, :, K // 2 : K],
                        op=mybir.AluOpType.add,
                    )
                    width = K // 2
                    while width > 2:
                        nc.vector.tensor_tensor(
                            out=h[:, :, 0 : width // 2],
                            in0=h[:, :, 0 : width // 2],
                            in1=h[:, :, width // 2 : width],
                            op=mybir.AluOpType.add,
                        )
                        width //= 2
                    # s in both pair lanes: out[:,i,j] = h[:,i,j] + h[:,i,1-j]
                    s2 = spool.tile([C, NCH // 2, 2], BF16, tag=f"s2{w}", name=f"s2{w}")
                    rev = bass.AP(
                        tensor=h.tensor,
                        offset=h.offset + 1,
                        ap=[h.ap[0], [h.ap[1][0], NCH // 2], [-1, 2]],
                    )
                    nc.vector.tensor_tensor(
                        out=s2, in0=h[:, :, 0:2], in1=rev, op=mybir.AluOpType.add
                    )
                # r2: reciprocal pairs keep the scale's broadcast AP packed (2x)
                r2 = spool.tile([C, NCH // 2, 2], BF16, tag=f"r2{w}", name=f"r2{w}")
                with nc.allow_low_precision(reason="softmax recip, tolerance 2e-2"):
                    nc.vector.reciprocal(r2, s2)
                for sname, lo, hi in SCALE_PLAN:
                    in1 = bass.AP(
                        tensor=r2.tensor,
                        offset=r2.offset + lo * r2.ap[1][0],
                        ap=[r2.ap[0], [r2.ap[1][0], hi - lo], [0, K // 2], [1, 2]],
                    )
                    eng[sname].tensor_tensor(
                        out=a[:, lo_w + lo : lo_w + hi, :],
                        in0=e[:, lo:hi, :],
                        in1=in1,
                        op=mybir.AluOpType.mult,
                    )

            # --- x transposes + psum->sbuf copies, in TWAVE-chunk waves ---
            xts = xtpool.tile([C, NCH, C], BF16, tag="xts")
            for tw in range(NTW):
                pt = pt_pool.tile([C, TWAVE, C], BF16, tag="pt")
                for j in range(TWAVE):
                    i = tw * TWAVE + j
                    nc.tensor.transpose(pt[:, j, :], in_=xsl(i), identity=ident_sb)
                for cname, lo, hi in COPY_PLAN[tw]:
                    dst = xts[:, tw * TWAVE + lo : tw * TWAVE + hi, :]
                    srcp = pt[:, lo:hi, :]
                    if cname == "scalar":
                        eng[cname].copy(out=dst, in_=srcp)
                    else:
                        eng[cname].tensor_copy(out=dst, in_=srcp)
            return a, xts

        def issue_back(ib, a, xts):
            """mm2 + asum accumulation and the psum drain for batch ib."""
            pv = pv_pool.tile([C, K], F32, tag="pv")
            pvs = pvs_pool.tile([K, 1], F32, tag="pvs")
            for i in range(NCH):
                a_chunk = a[:, i, :]
                nc.tensor.matmul(
                    pv,
                    lhsT=xts[:, i, :],
                    rhs=a_chunk,
                    start=(i == 0),
                    stop=(i == NCH - 1),
                )
                nc.tensor.matmul(
                    pvs,
                    lhsT=a_chunk,
                    rhs=ones_sb,
                    start=(i == 0),
                    stop=(i == NCH - 1),
                )
            outt = opool.tile([C, K + 1], F32, tag="o")
            if DRAIN_ENG == "scalar":
                nc.scalar.copy(out=outt[:, 0:K], in_=pv)
                nc.scalar.copy(out=outt[0:K, K : K + 1], in_=pvs)
            else:
                eng[DRAIN_ENG].tensor_copy(out=outt[:, 0:K], in_=pv)
                eng[DRAIN_ENG].tensor_copy(out=outt[0:K, K : K + 1], in_=pvs)
            nc.sync.dma_start(out=out_d[ib], in_=outt)

        pending = None
        for ib in range(BPC):
            front = issue_front(ib)
            if pending is not None:
                issue_back(ib - 1, *pending)
            pending = front
        issue_back(BPC - 1, *pending)


_NC_CACHE = None


def _get_nc():
    global _NC_CACHE
    if _NC_CACHE is None:
        nc = bacc.Bacc(
            "TRN2",
            target_bir_lowering=False,
            debug=False,
            num_devices=NCORES,
        )
        x_d = nc.dram_tensor("x", [BPC, C, HW], BF16, kind="ExternalInput").ap()
        w_d = nc.dram_tensor("w_t", [C, K], BF16, kind="ExternalInput").ap()
        ident_d = nc.dram_tensor("ident", [C, C], BF16, kind="ExternalInput").ap()
        out_d = nc.dram_tensor("out", [BPC, C, K + 1], F32, kind="ExternalOutput").ap()
        with tile.TileContext(nc) as tc:
            _netvlad_tile(tc, out_d, x_d, w_d, ident_d)
        nc.compile()
        _NC_CACHE = nc
    return _NC_CACHE


def _make_in_maps(x, conv_w):
    bf16 = ml_dtypes.bfloat16
    x_flat = np.ascontiguousarray(x.reshape(B, C, HW).astype(bf16))
    w_t = np.ascontiguousarray(conv_w.T.astype(bf16))  # [C, K]
    ident = np.eye(C, dtype=np.float32).astype(bf16)
    in_maps = []
    for core in range(NCORES):
        in_maps.append(
            {
                "x": x_flat[core * BPC : (core + 1) * BPC],
                "w_t": w_t,
                "ident": ident,
            }
        )
    return in_maps


def _run(in_maps, trace=False, **kwargs):
    nc = _get_nc()
    return bass_utils.run_bass_kernel_spmd(
        nc, in_maps, core_ids=list(range(NCORES)), trace=trace, **kwargs
    )


def _postprocess(raw, centroids):
    """raw: [B, C, K+1] = [vladT | asum] -> [B, K*C] normalized."""
    vlad = raw[:, :, :K].transpose(0, 2, 1) - raw[:, :K, K][:, :, None] * centroids[None]
    norms = np.sqrt((vlad * vlad).sum(axis=2, keepdims=True))
    vlad = vlad / np.maximum(norms, 1e-12)
    out = vlad.reshape(raw.shape[0], K * C)
    gn = np.sqrt((out * out).sum(axis=1, keepdims=True))
    return out / np.maximum(gn, 1e-12)


def kernel(x, conv_w, centroids):
    x = np.asarray(x)
    conv_w = np.asarray(conv_w)
    centroids = np.asarray(centroids, dtype=np.float32)
    res = _run(_make_in_maps(x, conv_w))
    raw = np.concatenate([r["out"] for r in res.results], axis=0)  # [B, C, K+1]
    return _postprocess(raw.astype(np.float32), centroids).astype(np.float32)
